# revision 16
# baseline (speedup 1.0000x reference)
"""DeformConv2d (B=8, C=128, H=W=64, K=3x3, pad 1, stride 1) on 8 trn2 NeuronCores.

Data-parallel over batch: core b handles image b. Transposed-gather design:
  - Host packs x into xpm[NE, 512] bf16: row i = 2x2 bilinear patch at padded
    pixel i as 4 channel-blocks [A, C, B, D] x 128 ch.
  - Host precomputes wrapped-16 indices (idxc) ordered so gather ordinal
    i = pos-within-quarter, and compact per-position corner weights
    wtr[p, (k,qt,b,j)] -- no weight broadcast DMA at all.
  - Per (quarter, tap): SWDGE dma_gather transpose=False (prepare_only,
    round-robin over 4 queues, explicit trigger) fetches 1024 rows of 1KB
    into [128 pos, 8 blk, (4 corner x 128 ch)] bf16. DVE multiplies by
    corner weights (free-dim stride-0 broadcast) and sums the 4 corners.
    PE transposes each 128-pos block to channel-major (PSUM), ACT copies
    back to SBUF, PE matmul accumulates 9 taps into psum[cout, 1024].
  - Tail per quarter: bias add -> fp32 out (contiguous store).
"""
import numpy as np
import ml_dtypes

B, CIN, H, W = 8, 128, 64, 64
COUT, KH, KW = 128, 3, 3
K = KH * KW
HO, WO = 64, 64
P = 128
NPOS = HO * WO               # 4096
PADR = 2
HP = H + 2 * PADR            # 68
WP = W + 2 * PADR            # 68
NE = HP * WP                 # 4624
ES = 4 * P                   # 512 bf16 per xpm row (1KB)
NQT = NPOS // 4              # 1024 positions per quarter
NB = NQT // P                # 8 position-blocks per quarter


def _build_kernel():
    import concourse.bacc as bacc
    import concourse.mybir as mybir
    import concourse.tile as tile
    import concourse.library_config as library_config

    nc = bacc.Bacc("TRN2", target_bir_lowering=False, debug=False,
                   num_devices=8, num_swdge_queues=4)
    f32, bf16, i16 = mybir.dt.float32, mybir.dt.bfloat16, mybir.dt.int16
    ALU = mybir.AluOpType

    xpm_d = nc.dram_tensor("xpm", [NE, ES], bf16, kind="ExternalInput")
    idxc_d = nc.dram_tensor("idxc", [P, K * 4 * 64], i16, kind="ExternalInput")
    wtr_d = nc.dram_tensor("wtr", [P, K * 4 * NB * 4], bf16, kind="ExternalInput")
    wmat_d = nc.dram_tensor("wmat", [P, K * COUT], bf16, kind="ExternalInput")
    bias_d = nc.dram_tensor("bias", [P, 1], f32, kind="ExternalInput")
    ident_d = nc.dram_tensor("ident", [P, P], bf16, kind="ExternalInput")
    out_d = nc.dram_tensor("out", [P, NPOS], f32, kind="ExternalOutput")

    NI = K * 4 * 64  # 2304

    with tile.TileContext(nc) as tc:
        with tc.tile_pool(name="const", bufs=1) as cpool, \
             tc.tile_pool(name="gath", bufs=6) as gapool, \
             tc.tile_pool(name="m1p", bufs=3) as m1pool, \
             tc.tile_pool(name="rp", bufs=3) as rpool, \
             tc.tile_pool(name="mtp", bufs=3) as mtpool, \
             tc.tile_pool(name="outp", bufs=2) as opool, \
             tc.tile_pool(name="ps", bufs=2, space="PSUM") as pspool:

            nc.gpsimd.load_library(library_config.mlp)

            idxc = cpool.tile([P, NI], i16)
            nc.sync.dma_start(out=idxc[:], in_=idxc_d.ap())
            wtr = cpool.tile([P, K * 4 * NB * 4], bf16)
            nc.scalar.dma_start(out=wtr[:], in_=wtr_d.ap())
            wmat = cpool.tile([P, K * COUT], bf16)
            nc.scalar.dma_start(out=wmat[:], in_=wmat_d.ap())
            bias = cpool.tile([P, 1], f32)
            nc.sync.dma_start(out=bias[:], in_=bias_d.ap())
            ident = cpool.tile([P, P], bf16)
            nc.sync.dma_start(out=ident[:], in_=ident_d.ap())

            rr = 0
            for qt in range(4):
                acc = pspool.tile([P, NQT], mybir.dt.float32, tag="acc")
                for k in range(K):
                    qn = rr % 4
                    g = gapool.tile([P, NB * ES], bf16, tag="g")
                    i0 = k * 256 + qt * 64
                    nc.gpsimd.dma_gather(
                        g[:].rearrange("p (b e) -> p b e", b=NB),
                        xpm_d.ap(), idxc[:, i0: i0 + 64],
                        num_idxs=NQT, num_idxs_reg=NQT,
                        elem_size=ES, transpose=False,
                        queue_num=qn, single_packet=False)

                    # weights: wtr[:, (k,qt,b,j)]; rows are (ch, corner)
                    # interleaved so every operand has unit innermost stride
                    # (j), keeping the DVE 2x 16-bit path alive; the weight
                    # broadcast over ch sits on a middle stride-0 dim.
                    wsl = wtr[:, (k * 4 + qt) * NB * 4:
                              (k * 4 + qt + 1) * NB * 4]
                    m1 = m1pool.tile([P, NB * ES], bf16, tag="m1")
                    nc.vector.tensor_tensor(
                        out=m1[:].rearrange("p (b e j) -> p b e j", b=NB, j=4),
                        in0=g[:].rearrange("p (b e j) -> p b e j", b=NB, j=4),
                        in1=wsl.rearrange("p (b j o) -> p b o j", b=NB, j=4,
                                          o=1).to_broadcast((P, NB, P, 4)),
                        op=ALU.mult)

                    # corner sum in two pairwise stages (tensor_reduce runs
                    # 1x here; two 1x adds of 2048+1024 elems are cheaper)
                    m1p = m1[:].rearrange("p (be t u) -> p be t u", t=2, u=2)
                    rp = rpool.tile([P, 2 * NQT], bf16, tag="rp")
                    nc.vector.tensor_tensor(
                        out=rp[:].rearrange("p (be t) -> p be t", t=2),
                        in0=m1p[:, :, :, 0], in1=m1p[:, :, :, 1], op=ALU.add)
                    rpv = rp[:].rearrange("p (be t) -> p be t", t=2)
                    m2 = rpool.tile([P, NQT], bf16, tag="m2")
                    nc.vector.tensor_tensor(
                        out=m2[:], in0=rpv[:, :, 0], in1=rpv[:, :, 1],
                        op=ALU.add)

                    # PE transpose per 128-pos block -> [ch, pos] in PSUM
                    pst = pspool.tile([P, NQT], bf16, tag="tr")
                    for b in range(NB):
                        nc.tensor.transpose(
                            pst[:, b * P: (b + 1) * P],
                            m2[:, b * P: (b + 1) * P], ident[:])
                    mt = mtpool.tile([P, NQT], bf16, tag="mt")
                    nc.scalar.copy(out=mt[:], in_=pst[:])

                    lhsT = wmat[:, k * COUT: (k + 1) * COUT]
                    for h in range(2):
                        nc.tensor.matmul(
                            acc[:, h * 512: (h + 1) * 512], lhsT,
                            mt[:, h * 512: (h + 1) * 512],
                            start=(k == 0), stop=(k == K - 1),
                            skip_group_check=True)
                    rr += 1

                ov = opool.tile([P, NQT], f32, tag="o")
                nc.vector.tensor_scalar(
                    out=ov[:], in0=acc[:], scalar1=bias[:, 0:1],
                    scalar2=None, op0=ALU.add)
                eng = nc.sync if qt % 2 == 0 else nc.scalar
                eng.dma_start(
                    out=out_d.ap()[:, qt * NQT: (qt + 1) * NQT], in_=ov[:])

    nc.compile()
    return nc


_NC_CACHE = None


def _host_inputs(x, offset, weight, bias):
    """Per-core input maps (core b <- batch b) + replicated constants."""
    wq = np.ascontiguousarray(weight, np.float32)  # [COUT, CIN, KH, KW]
    wmat = wq.reshape(COUT, CIN, K).transpose(1, 2, 0).reshape(CIN, K * COUT)
    wmat = np.ascontiguousarray(wmat).astype(ml_dtypes.bfloat16)
    bias_h = np.ascontiguousarray(bias, np.float32).reshape(P, 1)
    ident = np.eye(P, dtype=ml_dtypes.bfloat16)

    ho = (np.arange(NPOS, dtype=np.int32) // WO).astype(np.float32)
    wo = (np.arange(NPOS, dtype=np.int32) % WO).astype(np.float32)
    ky = (np.arange(K, dtype=np.int32) // 3 - 1).astype(np.float32)
    kx = (np.arange(K, dtype=np.int32) % 3 - 1).astype(np.float32)

    in_maps = []
    for b in range(B):
        img = np.ascontiguousarray(x[b], np.float32).transpose(1, 2, 0)
        XPf = np.zeros((HP, WP, P), np.float32)
        XPf[PADR:PADR + H, PADR:PADR + W] = img
        ext = np.vstack([XPf.reshape(NE, P), np.zeros((WP + 1, P), np.float32)])
        # row i = (ch, corner)-interleaved 2x2 patch: [.., x(i)_e, x(i+68)_e,
        # x(i+1)_e, x(i+69)_e, ..] so corner j is the unit-stride axis.
        xpm = np.stack(
            [ext[0:NE], ext[WP:NE + WP], ext[1:NE + 1], ext[WP + 1:NE + WP + 1]],
            axis=2).reshape(NE, ES).astype(ml_dtypes.bfloat16)  # [NE, 512]

        offb = np.ascontiguousarray(offset[b], np.float32).reshape(2 * K, NPOS)
        py = (ky[:, None] + ho[None, :]) + offb[0::2]   # [K, NPOS] f32
        px = (kx[:, None] + wo[None, :]) + offb[1::2]
        y0 = np.floor(py)
        x0 = np.floor(px)
        ly = py - y0
        lx = px - x0
        y0c = np.clip(y0, -PADR, 64.0)
        x0c = np.clip(x0, -PADR, 64.0)
        lin = ((y0c + PADR) * WP + (x0c + PADR)).astype(np.int16)  # [K, NPOS]

        # idxc[s + 16g, k*256 + qt*64 + t] = lin[k, qt*1024 + 16t + s]
        lin_r = lin.reshape(K, 4, 64, 16)             # k qt t s
        idx16 = lin_r.transpose(3, 0, 1, 2).reshape(16, K * 4 * 64)
        idxc = np.ascontiguousarray(np.tile(idx16, (8, 1)))  # [128, 2304]

        # wtr[p, ((k*4+qt)*NB + b)*4 + j] = w_j[k, qt*1024 + b*128 + p]
        w4 = np.stack([(1 - ly) * (1 - lx), ly * (1 - lx),
                       (1 - ly) * lx, ly * lx])        # j k pos
        w4r = w4.reshape(4, K, 4, NB, P)               # j k qt b p
        wtr = np.ascontiguousarray(
            w4r.transpose(4, 1, 2, 3, 0).reshape(P, K * 4 * NB * 4)
        ).astype(ml_dtypes.bfloat16)

        in_maps.append({
            "xpm": xpm,
            "idxc": idxc,
            "wtr": wtr,
            "wmat": wmat,
            "bias": bias_h,
            "ident": ident,
        })
    return in_maps


def kernel(x, offset, weight, bias):
    global _NC_CACHE
    from concourse.bass_utils import run_bass_kernel_spmd

    if _NC_CACHE is None:
        _NC_CACHE = _build_kernel()
    nc = _NC_CACHE
    in_maps = _host_inputs(x, offset, weight, bias)
    res = run_bass_kernel_spmd(nc, in_maps, list(range(B)))
    out = np.stack([res.results[b]["out"].reshape(COUT, HO, WO) for b in range(B)])
    return out.astype(np.float32)


if __name__ == "__main__":
    import sys
    d = np.load("/tmp/inputs.npz")
    if len(sys.argv) > 1 and sys.argv[1] == "sim":
        from concourse.bass_interp import CoreSim
        nc = _build_kernel()
        in_maps = _host_inputs(d["x"], d["offset"], d["weight"], d["bias"])
        sim = CoreSim(nc)
        for kk, vv in in_maps[0].items():
            sim.tensor(kk)[:] = vv
        sim.simulate()
        out = np.asarray(sim.tensor("out")).reshape(1, COUT, HO, WO)
        exp = np.load("/tmp/expected.npy")[0:1]
    else:
        out = kernel(d["x"], d["offset"], d["weight"], d["bias"])
        exp = np.load("/tmp/expected.npy")
    err = np.abs(out - exp)
    print("rel l2:", np.linalg.norm(out - exp) / np.linalg.norm(exp))
    print("absmax rel:", err.max() / np.abs(exp).max())


# revision 19
# speedup vs baseline: 1.2549x; 1.2549x over previous
"""DeformConv2d (B=8, C=128, H=W=64, K=3x3, pad 1, stride 1) on 8 trn2 NeuronCores.

Data-parallel over batch: core b handles image b. Transposed-gather design:
  - Host packs x into xpm[NE, 512] bf16: row i = 2x2 bilinear patch at padded
    pixel i as 4 channel-blocks [A, C, B, D] x 128 ch.
  - Host precomputes wrapped-16 indices (idxc) ordered so gather ordinal
    i = pos-within-quarter, and compact per-position corner weights
    wtr[p, (k,qt,b,j)] -- no weight broadcast DMA at all.
  - Per (quarter, tap): SWDGE dma_gather transpose=False (prepare_only,
    round-robin over 4 queues, explicit trigger) fetches 1024 rows of 1KB
    into [128 pos, 8 blk, (4 corner x 128 ch)] bf16. DVE multiplies by
    corner weights (free-dim stride-0 broadcast) and sums the 4 corners.
    PE transposes each 128-pos block to channel-major (PSUM), ACT copies
    back to SBUF, PE matmul accumulates 9 taps into psum[cout, 1024].
  - Tail per quarter: bias add -> fp32 out (contiguous store).
"""
import numpy as np
import ml_dtypes

B, CIN, H, W = 8, 128, 64, 64
COUT, KH, KW = 128, 3, 3
K = KH * KW
HO, WO = 64, 64
P = 128
NPOS = HO * WO               # 4096
PADR = 2
HP = H + 2 * PADR            # 68
WP = W + 2 * PADR            # 68
NE = HP * WP                 # 4624
ES = 4 * P                   # 512 bf16 per xpm row (1KB)
NQT = NPOS // 4              # 1024 positions per quarter
NB = NQT // P                # 8 position-blocks per quarter
BSP = 6                      # blocks corner-summed on DVE; the rest are
                             # summed on PE via regular matmuls vs identity
                             # (fp32 PSUM accumulation -- is_transpose bf16
                             # accumulation is broken on HW)


def _build_kernel():
    import concourse.bacc as bacc
    import concourse.mybir as mybir
    import concourse.tile as tile
    import concourse.library_config as library_config

    nc = bacc.Bacc("TRN2", target_bir_lowering=False, debug=False,
                   num_devices=8, num_swdge_queues=4)
    f32, bf16, i16 = mybir.dt.float32, mybir.dt.bfloat16, mybir.dt.int16
    ALU = mybir.AluOpType

    xpm_d = nc.dram_tensor("xpm", [NE, ES], bf16, kind="ExternalInput")
    idxc_d = nc.dram_tensor("idxc", [P, K * 4 * 64], i16, kind="ExternalInput")
    wtr_d = nc.dram_tensor("wtr", [P, K * 4 * NB * 4], bf16, kind="ExternalInput")
    wmat_d = nc.dram_tensor("wmat", [P, K * COUT], bf16, kind="ExternalInput")
    bias_d = nc.dram_tensor("bias", [P, 1], f32, kind="ExternalInput")
    ident_d = nc.dram_tensor("ident", [P, P], bf16, kind="ExternalInput")
    out_d = nc.dram_tensor("out", [P, NPOS], f32, kind="ExternalOutput")

    NI = K * 4 * 64  # 2304

    with tile.TileContext(nc) as tc:
        with tc.tile_pool(name="const", bufs=1) as cpool, \
             tc.tile_pool(name="gath", bufs=6) as gapool, \
             tc.tile_pool(name="m1p", bufs=3) as m1pool, \
             tc.tile_pool(name="rp", bufs=3) as rpool, \
             tc.tile_pool(name="mtp", bufs=3) as mtpool, \
             tc.tile_pool(name="outp", bufs=2) as opool, \
             tc.tile_pool(name="ps", bufs=2, space="PSUM") as pspool:

            nc.gpsimd.load_library(library_config.mlp)

            idxc = cpool.tile([P, NI], i16)
            nc.sync.dma_start(out=idxc[:], in_=idxc_d.ap())
            wtr = cpool.tile([P, K * 4 * NB * 4], bf16)
            nc.scalar.dma_start(out=wtr[:], in_=wtr_d.ap())
            wmat = cpool.tile([P, K * COUT], bf16)
            nc.scalar.dma_start(out=wmat[:], in_=wmat_d.ap())
            bias = cpool.tile([P, 1], f32)
            nc.sync.dma_start(out=bias[:], in_=bias_d.ap())
            ident = cpool.tile([P, P], bf16)
            nc.sync.dma_start(out=ident[:], in_=ident_d.ap())

            rr = 0
            for qt in range(4):
                acc = pspool.tile([P, NQT], mybir.dt.float32, tag="acc")
                for k in range(K):
                    qn = rr % 4
                    g = gapool.tile([P, NB * ES], bf16, tag="g")
                    i0 = k * 256 + qt * 64
                    nc.gpsimd.dma_gather(
                        g[:].rearrange("p (b e) -> p b e", b=NB),
                        xpm_d.ap(), idxc[:, i0: i0 + 64],
                        num_idxs=NQT, num_idxs_reg=NQT,
                        elem_size=ES, transpose=False,
                        queue_num=qn, single_packet=False)

                    # weights: wtr[:, (k,qt,b,j)]; rows are (ch, corner)
                    # interleaved so every operand has unit innermost stride
                    # (j), keeping the DVE 2x 16-bit path alive; the weight
                    # broadcast over ch sits on a middle stride-0 dim.
                    wsl = wtr[:, (k * 4 + qt) * NB * 4:
                              (k * 4 + qt + 1) * NB * 4]
                    m1 = m1pool.tile([P, NB * ES], bf16, tag="m1")
                    nc.vector.tensor_tensor(
                        out=m1[:].rearrange("p (b e j) -> p b e j", b=NB, j=4),
                        in0=g[:].rearrange("p (b e j) -> p b e j", b=NB, j=4),
                        in1=wsl.rearrange("p (b j o) -> p b o j", b=NB, j=4,
                                          o=1).to_broadcast((P, NB, P, 4)),
                        op=ALU.mult)

                    # corner sum, split across engines to balance the pipe:
                    # blocks < BSP reduce on DVE, blocks >= BSP skip DVE and
                    # let the PE sum the 4 corner slices by accumulating
                    # transpose-matmuls (lhsT=m1 slice, rhs=identity) into
                    # fp32 PSUM. All transposes are regular matmuls so the
                    # whole pst tile is fp32.
                    m2 = rpool.tile([P, NQT], bf16, tag="m2")
                    with nc.allow_low_precision(
                            reason="4-corner bf16 sum, same as bf16 adds"):
                        nc.vector.tensor_reduce(
                            out=m2[:, 0: BSP * P],
                            in_=m1[:, 0: BSP * ES].rearrange(
                                "p (be j) -> p be j", j=4),
                            axis=mybir.AxisListType.X, op=ALU.add)

                    pst = pspool.tile([P, NQT], mybir.dt.float32, tag="tr")
                    for b in range(BSP):
                        nc.tensor.matmul(
                            pst[:, b * P: (b + 1) * P],
                            m2[:, b * P: (b + 1) * P], ident[:],
                            start=True, stop=True, skip_group_check=True)
                    m1v = m1[:].rearrange("p (b e j) -> p b e j", b=NB, j=4)
                    for b in range(BSP, NB):
                        for j in range(4):
                            nc.tensor.matmul(
                                pst[:, b * P: (b + 1) * P],
                                m1v[:, b, :, j], ident[:],
                                start=(j == 0), stop=(j == 3),
                                skip_group_check=True)
                    mt = mtpool.tile([P, NQT], bf16, tag="mt")
                    nc.scalar.copy(out=mt[:], in_=pst[:])

                    lhsT = wmat[:, k * COUT: (k + 1) * COUT]
                    for h in range(2):
                        nc.tensor.matmul(
                            acc[:, h * 512: (h + 1) * 512], lhsT,
                            mt[:, h * 512: (h + 1) * 512],
                            start=(k == 0), stop=(k == K - 1),
                            skip_group_check=True)
                    rr += 1

                ov = opool.tile([P, NQT], f32, tag="o")
                nc.vector.tensor_scalar(
                    out=ov[:], in0=acc[:], scalar1=bias[:, 0:1],
                    scalar2=None, op0=ALU.add)
                eng = nc.sync if qt % 2 == 0 else nc.scalar
                eng.dma_start(
                    out=out_d.ap()[:, qt * NQT: (qt + 1) * NQT], in_=ov[:])

    nc.compile()
    return nc


_NC_CACHE = None


def _host_inputs(x, offset, weight, bias):
    """Per-core input maps (core b <- batch b) + replicated constants."""
    wq = np.ascontiguousarray(weight, np.float32)  # [COUT, CIN, KH, KW]
    wmat = wq.reshape(COUT, CIN, K).transpose(1, 2, 0).reshape(CIN, K * COUT)
    wmat = np.ascontiguousarray(wmat).astype(ml_dtypes.bfloat16)
    bias_h = np.ascontiguousarray(bias, np.float32).reshape(P, 1)
    ident = np.eye(P, dtype=ml_dtypes.bfloat16)

    ho = (np.arange(NPOS, dtype=np.int32) // WO).astype(np.float32)
    wo = (np.arange(NPOS, dtype=np.int32) % WO).astype(np.float32)
    ky = (np.arange(K, dtype=np.int32) // 3 - 1).astype(np.float32)
    kx = (np.arange(K, dtype=np.int32) % 3 - 1).astype(np.float32)

    in_maps = []
    for b in range(B):
        img = np.ascontiguousarray(x[b], np.float32).transpose(1, 2, 0)
        XPf = np.zeros((HP, WP, P), np.float32)
        XPf[PADR:PADR + H, PADR:PADR + W] = img
        ext = np.vstack([XPf.reshape(NE, P), np.zeros((WP + 1, P), np.float32)])
        # row i = (ch, corner)-interleaved 2x2 patch: [.., x(i)_e, x(i+68)_e,
        # x(i+1)_e, x(i+69)_e, ..] so corner j is the unit-stride axis.
        xpm = np.stack(
            [ext[0:NE], ext[WP:NE + WP], ext[1:NE + 1], ext[WP + 1:NE + WP + 1]],
            axis=2).reshape(NE, ES).astype(ml_dtypes.bfloat16)  # [NE, 512]

        offb = np.ascontiguousarray(offset[b], np.float32).reshape(2 * K, NPOS)
        py = (ky[:, None] + ho[None, :]) + offb[0::2]   # [K, NPOS] f32
        px = (kx[:, None] + wo[None, :]) + offb[1::2]
        y0 = np.floor(py)
        x0 = np.floor(px)
        ly = py - y0
        lx = px - x0
        y0c = np.clip(y0, -PADR, 64.0)
        x0c = np.clip(x0, -PADR, 64.0)
        lin = ((y0c + PADR) * WP + (x0c + PADR)).astype(np.int16)  # [K, NPOS]

        # idxc[s + 16g, k*256 + qt*64 + t] = lin[k, qt*1024 + 16t + s]
        lin_r = lin.reshape(K, 4, 64, 16)             # k qt t s
        idx16 = lin_r.transpose(3, 0, 1, 2).reshape(16, K * 4 * 64)
        idxc = np.ascontiguousarray(np.tile(idx16, (8, 1)))  # [128, 2304]

        # wtr[p, ((k*4+qt)*NB + b)*4 + j] = w_j[k, qt*1024 + b*128 + p]
        w4 = np.stack([(1 - ly) * (1 - lx), ly * (1 - lx),
                       (1 - ly) * lx, ly * lx])        # j k pos
        w4r = w4.reshape(4, K, 4, NB, P)               # j k qt b p
        wtr = np.ascontiguousarray(
            w4r.transpose(4, 1, 2, 3, 0).reshape(P, K * 4 * NB * 4)
        ).astype(ml_dtypes.bfloat16)

        in_maps.append({
            "xpm": xpm,
            "idxc": idxc,
            "wtr": wtr,
            "wmat": wmat,
            "bias": bias_h,
            "ident": ident,
        })
    return in_maps


def kernel(x, offset, weight, bias):
    global _NC_CACHE
    from concourse.bass_utils import run_bass_kernel_spmd

    if _NC_CACHE is None:
        _NC_CACHE = _build_kernel()
    nc = _NC_CACHE
    in_maps = _host_inputs(x, offset, weight, bias)
    res = run_bass_kernel_spmd(nc, in_maps, list(range(B)))
    out = np.stack([res.results[b]["out"].reshape(COUT, HO, WO) for b in range(B)])
    return out.astype(np.float32)


if __name__ == "__main__":
    import sys
    d = np.load("/tmp/inputs.npz")
    if len(sys.argv) > 1 and sys.argv[1] == "sim":
        from concourse.bass_interp import CoreSim
        nc = _build_kernel()
        in_maps = _host_inputs(d["x"], d["offset"], d["weight"], d["bias"])
        sim = CoreSim(nc)
        for kk, vv in in_maps[0].items():
            sim.tensor(kk)[:] = vv
        sim.simulate()
        out = np.asarray(sim.tensor("out")).reshape(1, COUT, HO, WO)
        exp = np.load("/tmp/expected.npy")[0:1]
    else:
        out = kernel(d["x"], d["offset"], d["weight"], d["bias"])
        exp = np.load("/tmp/expected.npy")
    err = np.abs(out - exp)
    print("rel l2:", np.linalg.norm(out - exp) / np.linalg.norm(exp))
    print("absmax rel:", err.max() / np.abs(exp).max())


# revision 20
# speedup vs baseline: 1.2683x; 1.0107x over previous
"""DeformConv2d (B=8, C=128, H=W=64, K=3x3, pad 1, stride 1) on 8 trn2 NeuronCores.

Data-parallel over batch: core b handles image b. Transposed-gather design:
  - Host packs x into xpm[NE, 512] bf16: row i = 2x2 bilinear patch at padded
    pixel i as 4 channel-blocks [A, C, B, D] x 128 ch.
  - Host precomputes wrapped-16 indices (idxc) ordered so gather ordinal
    i = pos-within-quarter, and compact per-position corner weights
    wtr[p, (k,qt,b,j)] -- no weight broadcast DMA at all.
  - Per (quarter, tap): SWDGE dma_gather transpose=False (prepare_only,
    round-robin over 4 queues, explicit trigger) fetches 1024 rows of 1KB
    into [128 pos, 8 blk, (4 corner x 128 ch)] bf16. DVE multiplies by
    corner weights (free-dim stride-0 broadcast) and sums the 4 corners.
    PE transposes each 128-pos block to channel-major (PSUM), ACT copies
    back to SBUF, PE matmul accumulates 9 taps into psum[cout, 1024].
  - Tail per quarter: bias add -> fp32 out (contiguous store).
"""
import numpy as np
import ml_dtypes

B, CIN, H, W = 8, 128, 64, 64
COUT, KH, KW = 128, 3, 3
K = KH * KW
HO, WO = 64, 64
P = 128
NPOS = HO * WO               # 4096
PADR = 2
HP = H + 2 * PADR            # 68
WP = W + 2 * PADR            # 68
NE = HP * WP                 # 4624
ES = 4 * P                   # 512 bf16 per xpm row (1KB)
NQT = NPOS // 4              # 1024 positions per quarter
NB = NQT // P                # 8 position-blocks per quarter
BSP = 6                      # blocks corner-summed on DVE; the rest are
                             # summed on PE via regular matmuls vs identity
                             # (fp32 PSUM accumulation -- is_transpose bf16
                             # accumulation is broken on HW)


def _build_kernel():
    import concourse.bacc as bacc
    import concourse.mybir as mybir
    import concourse.tile as tile
    import concourse.library_config as library_config

    nc = bacc.Bacc("TRN2", target_bir_lowering=False, debug=False,
                   num_devices=8, num_swdge_queues=4)
    f32, bf16, i16 = mybir.dt.float32, mybir.dt.bfloat16, mybir.dt.int16
    ALU = mybir.AluOpType

    xpm_d = nc.dram_tensor("xpm", [NE, ES], bf16, kind="ExternalInput")
    idxc_d = nc.dram_tensor("idxc", [P, K * 4 * 64], i16, kind="ExternalInput")
    wtr_d = nc.dram_tensor("wtr", [P, K * 4 * NB * 4], bf16, kind="ExternalInput")
    wmat_d = nc.dram_tensor("wmat", [P, K * COUT], bf16, kind="ExternalInput")
    bias_d = nc.dram_tensor("bias", [P, 1], f32, kind="ExternalInput")
    ident_d = nc.dram_tensor("ident", [P, P], bf16, kind="ExternalInput")
    out_d = nc.dram_tensor("out", [P, NPOS], f32, kind="ExternalOutput")

    NI = K * 4 * 64  # 2304

    with tile.TileContext(nc) as tc:
        with tc.tile_pool(name="const", bufs=1) as cpool, \
             tc.tile_pool(name="gath", bufs=7) as gapool, \
             tc.tile_pool(name="m1p", bufs=4) as m1pool, \
             tc.tile_pool(name="rp", bufs=4) as rpool, \
             tc.tile_pool(name="mtp", bufs=4) as mtpool, \
             tc.tile_pool(name="outp", bufs=2) as opool, \
             tc.tile_pool(name="ps", bufs=2, space="PSUM") as pspool:

            nc.gpsimd.load_library(library_config.mlp)

            idxc = cpool.tile([P, NI], i16)
            nc.sync.dma_start(out=idxc[:], in_=idxc_d.ap())
            wtr = cpool.tile([P, K * 4 * NB * 4], bf16)
            nc.scalar.dma_start(out=wtr[:], in_=wtr_d.ap())
            wmat = cpool.tile([P, K * COUT], bf16)
            nc.scalar.dma_start(out=wmat[:], in_=wmat_d.ap())
            bias = cpool.tile([P, 1], f32)
            nc.sync.dma_start(out=bias[:], in_=bias_d.ap())
            ident = cpool.tile([P, P], bf16)
            nc.sync.dma_start(out=ident[:], in_=ident_d.ap())

            rr = 0
            for qt in range(4):
                acc = pspool.tile([P, NQT], mybir.dt.float32, tag="acc")
                for k in range(K):
                    qn = rr % 4
                    g = gapool.tile([P, NB * ES], bf16, tag="g")
                    i0 = k * 256 + qt * 64
                    nc.gpsimd.dma_gather(
                        g[:].rearrange("p (b e) -> p b e", b=NB),
                        xpm_d.ap(), idxc[:, i0: i0 + 64],
                        num_idxs=NQT, num_idxs_reg=NQT,
                        elem_size=ES, transpose=False,
                        queue_num=qn, single_packet=False)

                    # weights: wtr[:, (k,qt,b,j)]; rows are (ch, corner)
                    # interleaved so every operand has unit innermost stride
                    # (j), keeping the DVE 2x 16-bit path alive; the weight
                    # broadcast over ch sits on a middle stride-0 dim.
                    wsl = wtr[:, (k * 4 + qt) * NB * 4:
                              (k * 4 + qt + 1) * NB * 4]
                    m1 = m1pool.tile([P, NB * ES], bf16, tag="m1")
                    nc.vector.tensor_tensor(
                        out=m1[:].rearrange("p (b e j) -> p b e j", b=NB, j=4),
                        in0=g[:].rearrange("p (b e j) -> p b e j", b=NB, j=4),
                        in1=wsl.rearrange("p (b j o) -> p b o j", b=NB, j=4,
                                          o=1).to_broadcast((P, NB, P, 4)),
                        op=ALU.mult)

                    # corner sum, split across engines to balance the pipe:
                    # blocks < BSP reduce on DVE, blocks >= BSP skip DVE and
                    # let the PE sum the 4 corner slices by accumulating
                    # transpose-matmuls (lhsT=m1 slice, rhs=identity) into
                    # fp32 PSUM. All transposes are regular matmuls so the
                    # whole pst tile is fp32.
                    m2 = rpool.tile([P, NQT], bf16, tag="m2")
                    with nc.allow_low_precision(
                            reason="4-corner bf16 sum, same as bf16 adds"):
                        nc.vector.tensor_reduce(
                            out=m2[:, 0: BSP * P],
                            in_=m1[:, 0: BSP * ES].rearrange(
                                "p (be j) -> p be j", j=4),
                            axis=mybir.AxisListType.X, op=ALU.add)

                    pst = pspool.tile([P, NQT], mybir.dt.float32, tag="tr")
                    for b in range(BSP):
                        nc.tensor.matmul(
                            pst[:, b * P: (b + 1) * P],
                            m2[:, b * P: (b + 1) * P], ident[:],
                            start=True, stop=True, skip_group_check=True)
                    m1v = m1[:].rearrange("p (b e j) -> p b e j", b=NB, j=4)
                    for b in range(BSP, NB):
                        for j in range(4):
                            nc.tensor.matmul(
                                pst[:, b * P: (b + 1) * P],
                                m1v[:, b, :, j], ident[:],
                                start=(j == 0), stop=(j == 3),
                                skip_group_check=True)
                    mt = mtpool.tile([P, NQT], bf16, tag="mt")
                    nc.scalar.copy(out=mt[:], in_=pst[:])

                    lhsT = wmat[:, k * COUT: (k + 1) * COUT]
                    for h in range(2):
                        nc.tensor.matmul(
                            acc[:, h * 512: (h + 1) * 512], lhsT,
                            mt[:, h * 512: (h + 1) * 512],
                            start=(k == 0), stop=(k == K - 1),
                            skip_group_check=True)
                    rr += 1

                ov = opool.tile([P, NQT], f32, tag="o")
                nc.scalar.add(out=ov[:], in_=acc[:], add=bias[:, 0:1])
                eng = nc.sync if qt % 2 == 0 else nc.scalar
                eng.dma_start(
                    out=out_d.ap()[:, qt * NQT: (qt + 1) * NQT], in_=ov[:])

    nc.compile()
    return nc


_NC_CACHE = None


def _host_inputs(x, offset, weight, bias):
    """Per-core input maps (core b <- batch b) + replicated constants."""
    wq = np.ascontiguousarray(weight, np.float32)  # [COUT, CIN, KH, KW]
    wmat = wq.reshape(COUT, CIN, K).transpose(1, 2, 0).reshape(CIN, K * COUT)
    wmat = np.ascontiguousarray(wmat).astype(ml_dtypes.bfloat16)
    bias_h = np.ascontiguousarray(bias, np.float32).reshape(P, 1)
    ident = np.eye(P, dtype=ml_dtypes.bfloat16)

    ho = (np.arange(NPOS, dtype=np.int32) // WO).astype(np.float32)
    wo = (np.arange(NPOS, dtype=np.int32) % WO).astype(np.float32)
    ky = (np.arange(K, dtype=np.int32) // 3 - 1).astype(np.float32)
    kx = (np.arange(K, dtype=np.int32) % 3 - 1).astype(np.float32)

    in_maps = []
    for b in range(B):
        img = np.ascontiguousarray(x[b], np.float32).transpose(1, 2, 0)
        XPf = np.zeros((HP, WP, P), np.float32)
        XPf[PADR:PADR + H, PADR:PADR + W] = img
        ext = np.vstack([XPf.reshape(NE, P), np.zeros((WP + 1, P), np.float32)])
        # row i = (ch, corner)-interleaved 2x2 patch: [.., x(i)_e, x(i+68)_e,
        # x(i+1)_e, x(i+69)_e, ..] so corner j is the unit-stride axis.
        xpm = np.stack(
            [ext[0:NE], ext[WP:NE + WP], ext[1:NE + 1], ext[WP + 1:NE + WP + 1]],
            axis=2).reshape(NE, ES).astype(ml_dtypes.bfloat16)  # [NE, 512]

        offb = np.ascontiguousarray(offset[b], np.float32).reshape(2 * K, NPOS)
        py = (ky[:, None] + ho[None, :]) + offb[0::2]   # [K, NPOS] f32
        px = (kx[:, None] + wo[None, :]) + offb[1::2]
        y0 = np.floor(py)
        x0 = np.floor(px)
        ly = py - y0
        lx = px - x0
        y0c = np.clip(y0, -PADR, 64.0)
        x0c = np.clip(x0, -PADR, 64.0)
        lin = ((y0c + PADR) * WP + (x0c + PADR)).astype(np.int16)  # [K, NPOS]

        # idxc[s + 16g, k*256 + qt*64 + t] = lin[k, qt*1024 + 16t + s]
        lin_r = lin.reshape(K, 4, 64, 16)             # k qt t s
        idx16 = lin_r.transpose(3, 0, 1, 2).reshape(16, K * 4 * 64)
        idxc = np.ascontiguousarray(np.tile(idx16, (8, 1)))  # [128, 2304]

        # wtr[p, ((k*4+qt)*NB + b)*4 + j] = w_j[k, qt*1024 + b*128 + p]
        w4 = np.stack([(1 - ly) * (1 - lx), ly * (1 - lx),
                       (1 - ly) * lx, ly * lx])        # j k pos
        w4r = w4.reshape(4, K, 4, NB, P)               # j k qt b p
        wtr = np.ascontiguousarray(
            w4r.transpose(4, 1, 2, 3, 0).reshape(P, K * 4 * NB * 4)
        ).astype(ml_dtypes.bfloat16)

        in_maps.append({
            "xpm": xpm,
            "idxc": idxc,
            "wtr": wtr,
            "wmat": wmat,
            "bias": bias_h,
            "ident": ident,
        })
    return in_maps


def kernel(x, offset, weight, bias):
    global _NC_CACHE
    from concourse.bass_utils import run_bass_kernel_spmd

    if _NC_CACHE is None:
        _NC_CACHE = _build_kernel()
    nc = _NC_CACHE
    in_maps = _host_inputs(x, offset, weight, bias)
    res = run_bass_kernel_spmd(nc, in_maps, list(range(B)))
    out = np.stack([res.results[b]["out"].reshape(COUT, HO, WO) for b in range(B)])
    return out.astype(np.float32)


if __name__ == "__main__":
    import sys
    d = np.load("/tmp/inputs.npz")
    if len(sys.argv) > 1 and sys.argv[1] == "sim":
        from concourse.bass_interp import CoreSim
        nc = _build_kernel()
        in_maps = _host_inputs(d["x"], d["offset"], d["weight"], d["bias"])
        sim = CoreSim(nc)
        for kk, vv in in_maps[0].items():
            sim.tensor(kk)[:] = vv
        sim.simulate()
        out = np.asarray(sim.tensor("out")).reshape(1, COUT, HO, WO)
        exp = np.load("/tmp/expected.npy")[0:1]
    else:
        out = kernel(d["x"], d["offset"], d["weight"], d["bias"])
        exp = np.load("/tmp/expected.npy")
    err = np.abs(out - exp)
    print("rel l2:", np.linalg.norm(out - exp) / np.linalg.norm(exp))
    print("absmax rel:", err.max() / np.abs(exp).max())


# revision 21
# speedup vs baseline: 1.3733x; 1.0828x over previous
"""DeformConv2d (B=8, C=128, H=W=64, K=3x3, pad 1, stride 1) on 8 trn2 NeuronCores.

Data-parallel over batch: core b handles image b. Transposed-gather design:
  - Host packs x into xpm[NE, 512] bf16: row i = 2x2 bilinear patch at padded
    pixel i as 4 channel-blocks [A, C, B, D] x 128 ch.
  - Host precomputes wrapped-16 indices (idxc) ordered so gather ordinal
    i = pos-within-quarter, and compact per-position corner weights
    wtr[p, (k,qt,b,j)] -- no weight broadcast DMA at all.
  - Per (quarter, tap): SWDGE dma_gather transpose=False (prepare_only,
    round-robin over 4 queues, explicit trigger) fetches 1024 rows of 1KB
    into [128 pos, 8 blk, (4 corner x 128 ch)] bf16. DVE multiplies by
    corner weights (free-dim stride-0 broadcast) and sums the 4 corners.
    PE transposes each 128-pos block to channel-major (PSUM), ACT copies
    back to SBUF, PE matmul accumulates 9 taps into psum[cout, 1024].
  - Tail per quarter: bias add -> fp32 out (contiguous store).
"""
import numpy as np
import ml_dtypes

B, CIN, H, W = 8, 128, 64, 64
COUT, KH, KW = 128, 3, 3
K = KH * KW
HO, WO = 64, 64
P = 128
NPOS = HO * WO               # 4096
PADR = 2
HP = H + 2 * PADR            # 68
WP = W + 2 * PADR            # 68
NE = HP * WP                 # 4624
ES = 4 * P                   # 512 bf16 per xpm row (1KB)
NQT = NPOS // 4              # 1024 positions per quarter
NB = NQT // P                # 8 position-blocks per quarter
BSP = 6                      # blocks corner-summed on DVE; the rest are
                             # summed on PE via regular matmuls vs identity
                             # (fp32 PSUM accumulation -- is_transpose bf16
                             # accumulation is broken on HW)


def _build_kernel():
    import concourse.bacc as bacc
    import concourse.mybir as mybir
    import concourse.tile as tile
    import concourse.library_config as library_config

    nc = bacc.Bacc("TRN2", target_bir_lowering=False, debug=False,
                   num_devices=8, num_swdge_queues=4)
    f32, bf16, i16 = mybir.dt.float32, mybir.dt.bfloat16, mybir.dt.int16
    ALU = mybir.AluOpType

    xpm_d = nc.dram_tensor("xpm", [NE, ES], bf16, kind="ExternalInput")
    idxc_d = nc.dram_tensor("idxc", [P, K * 4 * 64], i16, kind="ExternalInput")
    wtr_d = nc.dram_tensor("wtr", [P, K * 4 * NB * 4], bf16, kind="ExternalInput")
    wmat_d = nc.dram_tensor("wmat", [P, K * COUT], bf16, kind="ExternalInput")
    bias_d = nc.dram_tensor("bias", [P, 1], f32, kind="ExternalInput")
    ident_d = nc.dram_tensor("ident", [P, P], bf16, kind="ExternalInput")
    diag_d = nc.dram_tensor("diag", [P, 36 * (NB - BSP) * 4 * P], bf16,
                            kind="ExternalInput")
    out_d = nc.dram_tensor("out", [P, NPOS], f32, kind="ExternalOutput")

    NI = K * 4 * 64  # 2304

    with tile.TileContext(nc) as tc:
        with tc.tile_pool(name="const", bufs=1) as cpool, \
             tc.tile_pool(name="gath", bufs=7) as gapool, \
             tc.tile_pool(name="m1p", bufs=4) as m1pool, \
             tc.tile_pool(name="rp", bufs=4) as rpool, \
             tc.tile_pool(name="mtp", bufs=4) as mtpool, \
             tc.tile_pool(name="outp", bufs=2) as opool, \
             tc.tile_pool(name="dgp", bufs=3) as dgpool, \
             tc.tile_pool(name="ps", bufs=2, space="PSUM") as pspool:

            nc.gpsimd.load_library(library_config.mlp)

            idxc = cpool.tile([P, NI], i16)
            nc.sync.dma_start(out=idxc[:], in_=idxc_d.ap())
            wtr = cpool.tile([P, K * 4 * NB * 4], bf16)
            nc.scalar.dma_start(out=wtr[:], in_=wtr_d.ap())
            wmat = cpool.tile([P, K * COUT], bf16)
            nc.scalar.dma_start(out=wmat[:], in_=wmat_d.ap())
            bias = cpool.tile([P, 1], f32)
            nc.sync.dma_start(out=bias[:], in_=bias_d.ap())
            ident = cpool.tile([P, P], bf16)
            nc.sync.dma_start(out=ident[:], in_=ident_d.ap())

            rr = 0
            for qt in range(4):
                acc = pspool.tile([P, NQT], mybir.dt.float32, tag="acc")
                for k in range(K):
                    qn = rr % 4
                    g = gapool.tile([P, NB * ES], bf16, tag="g")
                    i0 = k * 256 + qt * 64
                    nc.gpsimd.dma_gather(
                        g[:].rearrange("p (b e) -> p b e", b=NB),
                        xpm_d.ap(), idxc[:, i0: i0 + 64],
                        num_idxs=NQT, num_idxs_reg=NQT,
                        elem_size=ES, transpose=False,
                        queue_num=qn, single_packet=False)

                    # weights: wtr[:, (k,qt,b,j)]; rows are (ch, corner)
                    # interleaved so every operand has unit innermost stride
                    # (j), keeping the DVE 2x 16-bit path alive; the weight
                    # broadcast over ch sits on a middle stride-0 dim.
                    # per-iter diag weight matrices for the PE-summed blocks
                    it = qt * K + k
                    NDG = (NB - BSP) * 4 * P
                    dgt = dgpool.tile([P, NDG], bf16, tag="dg")
                    deng = nc.sync if rr % 2 == 0 else nc.scalar
                    deng.dma_start(
                        out=dgt[:],
                        in_=diag_d.ap()[:, it * NDG: (it + 1) * NDG])

                    # DVE weights only the BSP DVE-reduced blocks (2x path)
                    wsl = wtr[:, (k * 4 + qt) * NB * 4:
                              (k * 4 + qt + 1) * NB * 4]
                    m1 = m1pool.tile([P, BSP * ES], bf16, tag="m1")
                    nc.vector.tensor_tensor(
                        out=m1[:].rearrange("p (b e j) -> p b e j", b=BSP,
                                            j=4),
                        in0=g[:, 0: BSP * ES].rearrange(
                            "p (b e j) -> p b e j", b=BSP, j=4),
                        in1=wsl.rearrange("p (b j o) -> p b o j", b=NB, j=4,
                                          o=1)[:, 0:BSP]
                            .to_broadcast((P, BSP, P, 4)),
                        op=ALU.mult)

                    # corner sum, split across engines to balance the pipe:
                    # blocks < BSP reduce on DVE, blocks >= BSP skip DVE and
                    # let the PE sum the 4 corner slices by accumulating
                    # transpose-matmuls (lhsT=m1 slice, rhs=identity) into
                    # fp32 PSUM. All transposes are regular matmuls so the
                    # whole pst tile is fp32.
                    m2 = rpool.tile([P, BSP * P], bf16, tag="m2")
                    with nc.allow_low_precision(
                            reason="4-corner bf16 sum, same as bf16 adds"):
                        nc.vector.tensor_reduce(
                            out=m2[:],
                            in_=m1[:].rearrange("p (be j) -> p be j", j=4),
                            axis=mybir.AxisListType.X, op=ALU.add)

                    pst = pspool.tile([P, NQT], mybir.dt.float32, tag="tr")
                    for b in range(BSP):
                        nc.tensor.matmul(
                            pst[:, b * P: (b + 1) * P],
                            m2[:, b * P: (b + 1) * P], ident[:],
                            start=True, stop=True, skip_group_check=True)
                    # PE-summed blocks: raw gathered corners as stationary,
                    # diag(w_j) as moving -> weight+transpose+sum in PSUM
                    gv = g[:].rearrange("p (b e j) -> p b e j", b=NB, j=4)
                    for b in range(BSP, NB):
                        for j in range(4):
                            dsl = ((b - BSP) * 4 + j) * P
                            nc.tensor.matmul(
                                pst[:, b * P: (b + 1) * P],
                                gv[:, b, :, j], dgt[:, dsl: dsl + P],
                                start=(j == 0), stop=(j == 3),
                                skip_group_check=True)
                    mt = mtpool.tile([P, NQT], bf16, tag="mt")
                    nc.scalar.copy(out=mt[:], in_=pst[:])

                    lhsT = wmat[:, k * COUT: (k + 1) * COUT]
                    for h in range(2):
                        nc.tensor.matmul(
                            acc[:, h * 512: (h + 1) * 512], lhsT,
                            mt[:, h * 512: (h + 1) * 512],
                            start=(k == 0), stop=(k == K - 1),
                            skip_group_check=True)
                    rr += 1

                ov = opool.tile([P, NQT], f32, tag="o")
                nc.scalar.add(out=ov[:], in_=acc[:], add=bias[:, 0:1])
                eng = nc.sync if qt % 2 == 0 else nc.scalar
                eng.dma_start(
                    out=out_d.ap()[:, qt * NQT: (qt + 1) * NQT], in_=ov[:])

    nc.compile()
    return nc


_NC_CACHE = None


def _host_inputs(x, offset, weight, bias):
    """Per-core input maps (core b <- batch b) + replicated constants."""
    wq = np.ascontiguousarray(weight, np.float32)  # [COUT, CIN, KH, KW]
    wmat = wq.reshape(COUT, CIN, K).transpose(1, 2, 0).reshape(CIN, K * COUT)
    wmat = np.ascontiguousarray(wmat).astype(ml_dtypes.bfloat16)
    bias_h = np.ascontiguousarray(bias, np.float32).reshape(P, 1)
    ident = np.eye(P, dtype=ml_dtypes.bfloat16)

    ho = (np.arange(NPOS, dtype=np.int32) // WO).astype(np.float32)
    wo = (np.arange(NPOS, dtype=np.int32) % WO).astype(np.float32)
    ky = (np.arange(K, dtype=np.int32) // 3 - 1).astype(np.float32)
    kx = (np.arange(K, dtype=np.int32) % 3 - 1).astype(np.float32)

    in_maps = []
    for b in range(B):
        img = np.ascontiguousarray(x[b], np.float32).transpose(1, 2, 0)
        XPf = np.zeros((HP, WP, P), np.float32)
        XPf[PADR:PADR + H, PADR:PADR + W] = img
        ext = np.vstack([XPf.reshape(NE, P), np.zeros((WP + 1, P), np.float32)])
        # row i = (ch, corner)-interleaved 2x2 patch: [.., x(i)_e, x(i+68)_e,
        # x(i+1)_e, x(i+69)_e, ..] so corner j is the unit-stride axis.
        xpm = np.stack(
            [ext[0:NE], ext[WP:NE + WP], ext[1:NE + 1], ext[WP + 1:NE + WP + 1]],
            axis=2).reshape(NE, ES).astype(ml_dtypes.bfloat16)  # [NE, 512]

        offb = np.ascontiguousarray(offset[b], np.float32).reshape(2 * K, NPOS)
        py = (ky[:, None] + ho[None, :]) + offb[0::2]   # [K, NPOS] f32
        px = (kx[:, None] + wo[None, :]) + offb[1::2]
        y0 = np.floor(py)
        x0 = np.floor(px)
        ly = py - y0
        lx = px - x0
        y0c = np.clip(y0, -PADR, 64.0)
        x0c = np.clip(x0, -PADR, 64.0)
        lin = ((y0c + PADR) * WP + (x0c + PADR)).astype(np.int16)  # [K, NPOS]

        # idxc[s + 16g, k*256 + qt*64 + t] = lin[k, qt*1024 + 16t + s]
        lin_r = lin.reshape(K, 4, 64, 16)             # k qt t s
        idx16 = lin_r.transpose(3, 0, 1, 2).reshape(16, K * 4 * 64)
        idxc = np.ascontiguousarray(np.tile(idx16, (8, 1)))  # [128, 2304]

        # wtr[p, ((k*4+qt)*NB + b)*4 + j] = w_j[k, qt*1024 + b*128 + p]
        w4 = np.stack([(1 - ly) * (1 - lx), ly * (1 - lx),
                       (1 - ly) * lx, ly * lx])        # j k pos
        w4r = w4.reshape(4, K, 4, NB, P)               # j k qt b p
        wtr = np.ascontiguousarray(
            w4r.transpose(4, 1, 2, 3, 0).reshape(P, K * 4 * NB * 4)
        ).astype(ml_dtypes.bfloat16)

        # diag[p, (it=(qt*K+k), bb, j)*128 + q] = w_j(...) * (q == p):
        # per-iteration diagonal weight matrices for the PE-summed blocks
        wsel = w4r[:, :, :, BSP:, :].transpose(4, 2, 1, 3, 0)  # p qt k bb j
        dd = np.zeros((P, 36, NB - BSP, 4, P), np.float32)
        pidx = np.arange(P)
        dd[pidx, :, :, :, pidx] = wsel.reshape(P, 36, NB - BSP, 4)
        diag = dd.reshape(P, 36 * (NB - BSP) * 4 * P).astype(ml_dtypes.bfloat16)

        in_maps.append({
            "xpm": xpm,
            "idxc": idxc,
            "wtr": wtr,
            "wmat": wmat,
            "bias": bias_h,
            "ident": ident,
            "diag": diag,
        })
    return in_maps


def kernel(x, offset, weight, bias):
    global _NC_CACHE
    from concourse.bass_utils import run_bass_kernel_spmd

    if _NC_CACHE is None:
        _NC_CACHE = _build_kernel()
    nc = _NC_CACHE
    in_maps = _host_inputs(x, offset, weight, bias)
    res = run_bass_kernel_spmd(nc, in_maps, list(range(B)))
    out = np.stack([res.results[b]["out"].reshape(COUT, HO, WO) for b in range(B)])
    return out.astype(np.float32)


if __name__ == "__main__":
    import sys
    d = np.load("/tmp/inputs.npz")
    if len(sys.argv) > 1 and sys.argv[1] == "sim":
        from concourse.bass_interp import CoreSim
        nc = _build_kernel()
        in_maps = _host_inputs(d["x"], d["offset"], d["weight"], d["bias"])
        sim = CoreSim(nc)
        for kk, vv in in_maps[0].items():
            sim.tensor(kk)[:] = vv
        sim.simulate()
        out = np.asarray(sim.tensor("out")).reshape(1, COUT, HO, WO)
        exp = np.load("/tmp/expected.npy")[0:1]
    else:
        out = kernel(d["x"], d["offset"], d["weight"], d["bias"])
        exp = np.load("/tmp/expected.npy")
    err = np.abs(out - exp)
    print("rel l2:", np.linalg.norm(out - exp) / np.linalg.norm(exp))
    print("absmax rel:", err.max() / np.abs(exp).max())
